# revision 40
# baseline (speedup 1.0000x reference)
"""Grouped per-sample MLP (conv1d groups=B) + GroupSwish + softmax, on 8 NeuronCores.

Data-parallel over the group/batch axis B=256: 32 groups per core,
processed as 8 quads of 4 groups packed into the 128-partition dim.

Per group g: h = W1[g] @ x[g] + b1[g]; GroupSwish; o = W2[g] @ h + b2[g];
softmax over the flattened [C*L] logits.

Key design points (vs. the fp32r per-group baseline at ~260us):
  - x and W1 are marshaled to fp16 host-side: halves HBM traffic (the
    dominant cost; x alone is 25.7MB/core in fp16). fp16 matmul error
    ~1e-3 rel, far inside the 2e-2 gate.
  - X=784 is split as 7 K-chunks of 112 so every chunk is uniform and the
    x DMA is one contiguous 28KB run per partition (112 partitions).
  - A quad of 4 groups shares each [128, L] tile: group j owns partitions
    32j..32j+32. W1/W2 matmuls are col-tiled (tile_position auto-derived
    from PSUM base partition) so the 4 groups' matmuls run concurrently
    in the PE array; ACT/DVE ops process 4 groups per instruction.
  - W2 is padded to [Z, 32] with zeros so all 128 partitions of the
    logits PSUM are written (pad rows get exp(-30) ~ 0).
  - Softmax cross-partition sum / broadcast via tiny matmuls against a
    [128,4] mask and a [4,128] select matrix.
  - softplus(beta), b1 folding and W2/1.1 folding are done host-side.
"""

import os
import ml_dtypes
import numpy as np
from contextlib import ExitStack

import concourse.mybir as mybir
import concourse.tile as tile
from concourse import bacc
from concourse.bass_utils import run_bass_kernel_spmd

B, X, Z, C, L = 256, 784, 32, 10, 512
NCORE = 8
GPC = B // NCORE  # 32 groups per core
NQ = GPC // 4  # 8 quads per core
KC = 112  # K-chunk size (7 * 112 = 784)
NCH = 7
P = 128
F32 = mybir.dt.float32
F16 = mybir.dt.float16
F8 = mybir.dt.float8e4
BF16 = mybir.dt.bfloat16

DEFAULT_CFG = dict(
    x_bufs=4,
    w_bufs=3,
    s_bufs=3,
    h_bufs=2,
    o_bufs=2,
    x_layout="cc",  # "jp": j-split halves, 14KB descs; "cc": c-split, 2KB descs
    x_engines=("sync", "gpsimd"),
    w_engine="sync",
    out_engine="gpsimd",
    out2_engine="sync",
    const_engine="gpsimd",
)

_CACHE: dict = {}


def _eng(nc, name):
    return getattr(nc, name)


def _build(cfg=DEFAULT_CFG):
    nc = bacc.Bacc("TRN2", target_bir_lowering=False, debug=False)

    # x split into two DMAs per quad so W1 can start on the first half
    # while the second streams. "jp": halves = groups (j01, j23), one 14KB
    # run per partition. "cc": halves = chunks (c0-3, c4-6), 2KB runs.
    if cfg["x_layout"] == "jp":
        xq = nc.dram_tensor(
            "xq", [NQ * 2, KC, 2 * NCH * L], F8, kind="ExternalInput"
        ).ap()
    else:
        xq = nc.dram_tensor(
            "xq", [NQ, NCH, KC, 4 * L], F8, kind="ExternalInput"
        ).ap()
    w1q = nc.dram_tensor(
        "w1q", [KC, NQ * 4 * NCH * Z], F8, kind="ExternalInput"
    ).ap()
    # w2q[32j+z, 32q+m] = W2[4q+j, m, z]/1.1 (m<C), 0 for m>=C
    w2q = nc.dram_tensor("w2q", [P, NQ * 32], F16, kind="ExternalInput").ap()
    b1q = nc.dram_tensor("b1q", [P, NQ], F32, kind="ExternalInput").ap()
    sphq = nc.dram_tensor("sphq", [P, NQ], F32, kind="ExternalInput").ap()
    spb1hq = nc.dram_tensor("spb1hq", [P, NQ], F32, kind="ExternalInput").ap()
    b2q = nc.dram_tensor("b2q", [P, NQ], F32, kind="ExternalInput").ap()
    # maskb[p, m] = 1 iff p//32 == m//32 and p%32 < C: one matmul turns the
    # per-partition exp sums into per-partition group totals (pad rows get
    # the same total, keeping reciprocal finite).
    maskb = nc.dram_tensor("maskb", [P, P], F32, kind="ExternalInput").ap()
    out = nc.dram_tensor("out", [GPC, C, L], BF16, kind="ExternalOutput").ap()

    with tile.TileContext(nc) as tc, ExitStack() as ctx:
        consts = ctx.enter_context(tc.tile_pool(name="consts", bufs=1))
        xpool = ctx.enter_context(tc.tile_pool(name="x", bufs=2 * cfg["x_bufs"]))
        spool = ctx.enter_context(tc.tile_pool(name="act", bufs=cfg["s_bufs"]))
        hps = ctx.enter_context(
            tc.tile_pool(name="hps", bufs=cfg["h_bufs"], space="PSUM")
        )
        ops = ctx.enter_context(
            tc.tile_pool(name="ops", bufs=cfg["o_bufs"], space="PSUM")
        )
        tps = ctx.enter_context(tc.tile_pool(name="tps", bufs=2, space="PSUM"))

        ce = _eng(nc, cfg["const_engine"])
        xes = [_eng(nc, e) for e in cfg["x_engines"]]
        we = _eng(nc, cfg["w_engine"])
        oe = _eng(nc, cfg["out_engine"])
        o2e = _eng(nc, cfg["out2_engine"])

        # all of W1 stays resident (14.3KB/partition) -> W1 matmuls gate on
        # the x DMA semaphore only
        w1t = consts.tile([KC, NQ * 4 * NCH * Z], F8, name="w1t")
        we.dma_start(w1t[:], w1q)
        w2t = consts.tile([P, NQ * 32], F16, name="w2t")
        ce.dma_start(w2t[:], w2q)
        b1t = consts.tile([P, NQ], F32, name="b1t")
        ce.dma_start(b1t[:], b1q)
        spht = consts.tile([P, NQ], F32, name="spht")
        ce.dma_start(spht[:], sphq)
        spb1ht = consts.tile([P, NQ], F32, name="spb1ht")
        ce.dma_start(spb1ht[:], spb1hq)
        b2t = consts.tile([P, NQ], F32, name="b2t")
        ce.dma_start(b2t[:], b2q)
        maskt = consts.tile([P, P], F32, name="maskt")
        ce.dma_start(maskt[:], maskb)

        # Software-pipelined emission: per iteration q the PE stream is
        # [28x W1(q)] [4x W2(q-1)] [tot(q-2), invb(q-2)] so every
        # cross-engine dependency (swish from DVE, exp sums from ACT,
        # reciprocal from DVE) has a quad of slack before the PE needs it.
        hqs, swishes, expos, esums, invcs = {}, {}, {}, {}, {}

        def w1s(q, j, c):
            k = (q * 4 + j) * NCH + c
            return w1t[:, k * Z : (k + 1) * Z]

        def stage1(q):
            """x loads (two halves), W1 matmuls for quad q."""
            hq = hps.tile([P, L], F32, tag="h", name=f"h{q}")
            hqs[q] = hq
            if cfg["x_layout"] == "jp":
                xts = []
                for h in range(2):
                    xt = xpool.tile(
                        [KC, 2 * NCH * L], F8, tag="xt", name=f"xt{q}_{h}"
                    )
                    xes[h % len(xes)].dma_start(xt[:], xq[2 * q + h])
                    xts.append(xt)
                for j in range(4):
                    xt = xts[j // 2]
                    for c in range(NCH):
                        k = (j % 2) * NCH + c
                        nc.tensor.matmul(
                            hq[32 * j : 32 * j + 32, :],
                            w1s(q, j, c),
                            xt[:, k * L : (k + 1) * L],
                            start=(c == 0),
                            stop=(c == NCH - 1),
                            tile_position=(0, 32 * j),
                        )
            else:
                # c-split: free layout (c, j, l); MMs c-outer so the first
                # 16 run off half A. Interleaved accumulation groups on
                # disjoint partition ranges -> skip the group check.
                splits = [(0, 3), (3, 5), (5, 7)] if len(xes) > 2 else [
                    (0, 4), (4, 7)
                ]
                xts = []
                for si, (c0, c1) in enumerate(splits):
                    nchunks = c1 - c0
                    xt = xpool.tile(
                        [KC, nchunks * 4 * L], F8, tag=f"xt{si}", name=f"x{q}_{si}"
                    )
                    xes[si % len(xes)].dma_start(
                        xt[:].rearrange("p (c r) -> p c r", c=nchunks),
                        xq[q, c0:c1].rearrange("c p r -> p c r"),
                    )
                    xts.append(xt)
                for c in range(NCH):
                    si = next(i for i, (c0, c1) in enumerate(splits) if c < c1)
                    xt, cc = xts[si], c - splits[si][0]
                    for j in range(4):
                        nc.tensor.matmul(
                            hq[32 * j : 32 * j + 32, :],
                            w1s(q, j, c),
                            xt[:, (cc * 4 + j) * L : (cc * 4 + j + 1) * L],
                            start=(c == 0),
                            stop=(c == NCH - 1),
                            tile_position=(0, 32 * j),
                            skip_group_check=True,
                        )

        def stage_swish(q):
            """GroupSwish for quad q: ((h+b1)*0.5) * (1 + tanh(sp*(h+b1)/2))."""
            hq = hqs.pop(q)
            t = spool.tile([P, L], F32, tag="t", name=f"t{q}")
            nc.scalar.activation(
                t[:],
                hq[:],
                mybir.ActivationFunctionType.Tanh,
                bias=spb1ht[:, q : q + 1],
                scale=spht[:, q : q + 1],
            )
            u = spool.tile([P, L], F32, tag="u", name=f"u{q}")
            nc.vector.tensor_scalar(
                u[:],
                hq[:],
                b1t[:, q : q + 1],
                0.5,
                op0=mybir.AluOpType.add,
                op1=mybir.AluOpType.mult,
            )
            sw = spool.tile([P, L], F16, tag="sw", name=f"sw{q}")
            nc.vector.scalar_tensor_tensor(
                sw[:],
                t[:],
                1.0,
                u[:],
                op0=mybir.AluOpType.add,
                op1=mybir.AluOpType.mult,
            )
            swishes[q] = sw

        def stage2(q):
            """W2 matmuls + exp for quad q (emitted one quad later)."""
            sw = swishes.pop(q)
            o = ops.tile([P, L], F32, tag="o", name=f"o{q}")
            for j in range(4):
                nc.tensor.matmul(
                    o[32 * j : 32 * j + 32, :],
                    w2t[32 * j : 32 * j + 32, q * 32 : (q + 1) * 32],
                    sw[32 * j : 32 * j + 32, :],
                    start=True,
                    stop=True,
                    tile_position=(32 * j, 32 * j),
                )
            expo = spool.tile([P, L], F32, tag="expo", name=f"e{q}")
            esum = spool.tile([P, 1], F32, tag="esum", name=f"es{q}")
            nc.scalar.activation(
                expo[:],
                o[:],
                mybir.ActivationFunctionType.Exp,
                bias=b2t[:, q : q + 1],
                scale=1.0,
                accum_out=esum[:],
            )
            expos[q] = expo
            esums[q] = esum

        def stage3a(q):
            """Per-group exp totals + reciprocal (two quads later)."""
            esum = esums.pop(q)
            tot = tps.tile([P, 1], F32, tag="tot", name=f"tot{q}")
            nc.tensor.matmul(tot[:], maskt[:], esum[:], start=True, stop=True)
            invc = spool.tile([P, 1], F32, tag="invc", name=f"ic{q}")
            nc.vector.reciprocal(invc[:], tot[:])
            invcs[q] = invc

        def stage3b(q):
            """Normalize + store (two quads later, after stage3a)."""
            invc = invcs.pop(q)
            expo = expos.pop(q)
            res = spool.tile([P, L], BF16, tag="res", name=f"r{q}")
            nc.vector.tensor_scalar_mul(res[:], expo[:], invc[:])
            for j in range(4):
                e = oe if j < 2 else o2e
                e.dma_start(out[4 * q + j], res[32 * j : 32 * j + C, :])

        for q in range(NQ + 2):
            if q < NQ:
                stage1(q)
                stage_swish(q)
            if 1 <= q <= NQ:
                stage2(q - 1)
            if q >= 2:
                stage3a(q - 2)
                stage3b(q - 2)

    nc.compile()
    return nc


def _marshal(x, W1, b1, beta, W2, b2, cfg=DEFAULT_CFG):
    """Full inputs -> list of per-core input dicts."""
    # x: [1, B*X, L] -> [B, 7, 112, L] (g, c, p, l)
    xg = np.asarray(x, dtype=np.float32).reshape(B, NCH, KC, L)
    w1T = np.asarray(W1, dtype=np.float32).transpose(0, 2, 1)  # [B, X, Z]
    w1g = w1T.reshape(B, NCH, KC, Z)  # (g, c, p, z)
    w2s = (np.asarray(W2, dtype=np.float32) * np.float32(1.0 / 1.1))  # [B, C, Z]
    b1f = np.asarray(b1, dtype=np.float32)  # [B, Z]
    b2f = np.asarray(b2, dtype=np.float32)  # [B, C]
    bf = np.asarray(beta, dtype=np.float32)  # [B]
    sph = np.log1p(np.exp(bf)) * np.float32(0.5)  # softplus(beta)/2

    pp = np.arange(P)
    maskb = (
        (pp[:, None] // 32 == pp[None, :] // 32) & (pp[:, None] % 32 < C)
    ).astype(np.float32)

    in_maps = []
    for core in range(NCORE):
        s = slice(core * GPC, (core + 1) * GPC)
        if cfg["x_layout"] == "jp":
            # xq[2q+h, p, j2, c, l] = x[4q+2h+j2, 112c+p, l]
            xc = xg[s].reshape(NQ, 2, 2, NCH, KC, L)
            xqm = (
                xc.transpose(0, 1, 4, 2, 3, 5)
                .astype(ml_dtypes.float8_e4m3)
                .reshape(NQ * 2, KC, 2 * NCH * L)
            )
        else:
            # xq[q, c, p, j, l] = x[4q+j, 112c+p, l]
            xc = xg[s].reshape(NQ, 4, NCH, KC, L)
            xqm = (
                xc.transpose(0, 2, 3, 1, 4)
                .astype(ml_dtypes.float8_e4m3)
                .reshape(NQ, NCH, KC, 4 * L)
            )
        # w1q[p, ((q*4+j)*7+c)*Z+z] = W1T[4q+j, 112c+p, z]
        wc = w1g[s].reshape(NQ, 4, NCH, KC, Z)
        w1qm = (
            wc.transpose(3, 0, 1, 2, 4)
            .astype(ml_dtypes.float8_e4m3)
            .reshape(KC, NQ * 4 * NCH * Z)
        )
        # w2q[32j+z, 32q+m] = W2[4q+j, m, z]/1.1 (m<C), else 0
        w2c = w2s[s].reshape(NQ, 4, C, Z)  # (q, j, m, z)
        w2qm = np.zeros((4, Z, NQ, 32), np.float16)
        w2qm[:, :, :, :C] = w2c.transpose(1, 3, 0, 2)
        w2qm = w2qm.reshape(P, NQ * 32)
        # per-partition scalars: [32j+z, q]
        b1qm = np.ascontiguousarray(
            b1f[s].reshape(NQ, 4, Z).transpose(1, 2, 0)
        ).reshape(P, NQ)
        sphqm = np.ascontiguousarray(
            np.broadcast_to(
                sph[s].reshape(NQ, 4).T[:, None, :], (4, Z, NQ)
            )
        ).reshape(P, NQ)
        spb1hqm = sphqm * b1qm
        b2qm = np.full((4, 32, NQ), -30.0, np.float32)
        b2qm[:, :C, :] = b2f[s].reshape(NQ, 4, C).transpose(1, 2, 0)
        b2qm = b2qm.reshape(P, NQ)
        in_maps.append(
            {
                "xq": xqm,
                "w1q": w1qm,
                "w2q": w2qm,
                "b1q": b1qm,
                "sphq": sphqm,
                "spb1hq": spb1hqm,
                "b2q": b2qm,
                "maskb": maskb,
            }
        )
    return in_maps


def _run(in_maps, cfg=DEFAULT_CFG, trace=False, tmpdir=None):
    key = str(sorted(cfg.items()))
    if key not in _CACHE:
        _CACHE[key] = _build(cfg)
    return run_bass_kernel_spmd(
        _CACHE[key],
        in_maps,
        core_ids=list(range(NCORE)),
        trace=trace,
        tmpdir=tmpdir,
    )


_LAST = {}


def kernel(x, W1, b1, beta, W2, b2):
    cfg = dict(DEFAULT_CFG)
    ov = os.environ.get("KERNEL_CFG")
    if ov:
        for kv in ov.split(","):
            k, v = kv.split("=")
            cfg[k] = type(DEFAULT_CFG[k])(eval(v)) if not isinstance(
                DEFAULT_CFG[k], str
            ) else v
    in_maps = _marshal(x, W1, b1, beta, W2, b2, cfg)
    trace = bool(os.environ.get("KERNEL_TRACE"))
    r = _run(in_maps, cfg, trace=trace, tmpdir=os.environ.get("KERNEL_TRACE_DIR"))
    _LAST["results"] = r
    outs = [
        r.results[c]["out"].astype(np.float32).reshape(GPC, C * L)
        for c in range(NCORE)
    ]
    return np.concatenate(outs, axis=0)


# revision 42
# speedup vs baseline: 1.0507x; 1.0507x over previous
"""Grouped per-sample MLP (conv1d groups=B) + GroupSwish + softmax, on 8 NeuronCores.

Data-parallel over the group/batch axis B=256: 32 groups per core,
processed as 8 quads of 4 groups packed into the 128-partition dim.

Per group g: h = W1[g] @ x[g] + b1[g]; GroupSwish; o = W2[g] @ h + b2[g];
softmax over the flattened [C*L] logits.

The kernel is HBM-stream-bound; every design choice minimizes bytes and
keeps the x stream uninterrupted (~250 GB/s/core under 8-core load):
  - x and W1 ship as fp8e4m3, W2/swish as fp16, out as bf16 (upcast on
    host). End-to-end rel err ~9e-3 vs the 2e-2 gate. 13.3MB/core total.
  - X=784 is split as 7 K-chunks of 112 (uniform K, partitions 0..111).
    Per quad, x is loaded in two DMAs (chunks 0-3 / 4-6, 2KB runs) on two
    queues (sync HWDGE + gpsimd SWDGE) so each SDMA engine keeps two
    packet streams in flight; W1 matmuls start on the first piece.
  - A quad of 4 groups shares each [128, L] tile: group j owns partitions
    32j..32j+32. W1/W2 matmuls are col-tiled via tile_position so the 4
    groups' matmuls run concurrently in the PE array; ACT/DVE ops process
    4 groups per instruction.
  - All of W1 stays resident in SBUF (loaded once) so W1 matmuls gate on
    the x semaphore only.
  - Software-pipelined emission: per iteration q the PE stream is
    [28x W1(q)] [4x W2(q-1)] [tot(q-2)] so every cross-engine dependency
    (swish from DVE, exp sums from ACT, reciprocal from DVE) has a quad
    of slack before the PE needs it; output stores split across two
    queues.
  - W2 is padded to [Z, 32] with zeros so all 128 partitions of the
    logits PSUM are written (pad rows get exp(-30) ~ 0). Softmax
    normalization via one [128,128] block-mask matmul that yields
    per-partition group totals directly.
  - softplus(beta), b1 folding and W2/1.1 folding are done host-side.
"""

import os
import ml_dtypes
import numpy as np
from contextlib import ExitStack

import concourse.mybir as mybir
import concourse.tile as tile
from concourse import bacc
from concourse.bass_utils import run_bass_kernel_spmd

B, X, Z, C, L = 256, 784, 32, 10, 512
NCORE = 8
GPC = B // NCORE  # 32 groups per core
NQ = GPC // 4  # 8 quads per core
KC = 112  # K-chunk size (7 * 112 = 784)
NCH = 7
P = 128
F32 = mybir.dt.float32
F16 = mybir.dt.float16
F8 = mybir.dt.float8e4
BF16 = mybir.dt.bfloat16

DEFAULT_CFG = dict(
    x_bufs=5,
    w_bufs=3,
    s_bufs=4,
    h_bufs=2,
    o_bufs=2,
    x_layout="cc",  # "jp": j-split halves, 14KB descs; "cc": c-split, 2KB descs
    x_engines=("sync", "gpsimd"),
    w_engine="sync",
    out_engine="gpsimd",
    out2_engine="sync",
    const_engine="gpsimd",
)

_CACHE: dict = {}


def _eng(nc, name):
    return getattr(nc, name)


def _build(cfg=DEFAULT_CFG):
    nc = bacc.Bacc("TRN2", target_bir_lowering=False, debug=False)

    # x split into two DMAs per quad so W1 can start on the first half
    # while the second streams. "jp": halves = groups (j01, j23), one 14KB
    # run per partition. "cc": halves = chunks (c0-3, c4-6), 2KB runs.
    if cfg["x_layout"] == "jp":
        xq = nc.dram_tensor(
            "xq", [NQ * 2, KC, 2 * NCH * L], F8, kind="ExternalInput"
        ).ap()
    else:
        xq = nc.dram_tensor(
            "xq", [NQ, NCH, KC, 4 * L], F8, kind="ExternalInput"
        ).ap()
    w1q = nc.dram_tensor(
        "w1q", [KC, NQ * 4 * NCH * Z], F8, kind="ExternalInput"
    ).ap()
    # w2q[32j+z, 32q+m] = W2[4q+j, m, z]/1.1 (m<C), 0 for m>=C
    w2q = nc.dram_tensor("w2q", [P, NQ * 32], F16, kind="ExternalInput").ap()
    b1q = nc.dram_tensor("b1q", [P, NQ], F32, kind="ExternalInput").ap()
    sphq = nc.dram_tensor("sphq", [P, NQ], F32, kind="ExternalInput").ap()
    spb1hq = nc.dram_tensor("spb1hq", [P, NQ], F32, kind="ExternalInput").ap()
    b2q = nc.dram_tensor("b2q", [P, NQ], F32, kind="ExternalInput").ap()
    # maskb[p, m] = 1 iff p//32 == m//32 and p%32 < C: one matmul turns the
    # per-partition exp sums into per-partition group totals (pad rows get
    # the same total, keeping reciprocal finite).
    maskb = nc.dram_tensor("maskb", [P, P], F32, kind="ExternalInput").ap()
    out = nc.dram_tensor("out", [GPC, C, L], BF16, kind="ExternalOutput").ap()

    with tile.TileContext(nc) as tc, ExitStack() as ctx:
        consts = ctx.enter_context(tc.tile_pool(name="consts", bufs=1))
        xpool = ctx.enter_context(tc.tile_pool(name="x", bufs=2 * cfg["x_bufs"]))
        spool = ctx.enter_context(tc.tile_pool(name="act", bufs=cfg["s_bufs"]))
        hps = ctx.enter_context(
            tc.tile_pool(name="hps", bufs=cfg["h_bufs"], space="PSUM")
        )
        ops = ctx.enter_context(
            tc.tile_pool(name="ops", bufs=cfg["o_bufs"], space="PSUM")
        )
        tps = ctx.enter_context(tc.tile_pool(name="tps", bufs=2, space="PSUM"))

        ce = _eng(nc, cfg["const_engine"])
        xes = [_eng(nc, e) for e in cfg["x_engines"]]
        we = _eng(nc, cfg["w_engine"])
        oe = _eng(nc, cfg["out_engine"])
        o2e = _eng(nc, cfg["out2_engine"])

        # all of W1 stays resident (14.3KB/partition) -> W1 matmuls gate on
        # the x DMA semaphore only
        w1t = consts.tile([KC, NQ * 4 * NCH * Z], F8, name="w1t")
        we.dma_start(w1t[:], w1q)
        w2t = consts.tile([P, NQ * 32], F16, name="w2t")
        ce.dma_start(w2t[:], w2q)
        b1t = consts.tile([P, NQ], F32, name="b1t")
        ce.dma_start(b1t[:], b1q)
        spht = consts.tile([P, NQ], F32, name="spht")
        ce.dma_start(spht[:], sphq)
        spb1ht = consts.tile([P, NQ], F32, name="spb1ht")
        ce.dma_start(spb1ht[:], spb1hq)
        b2t = consts.tile([P, NQ], F32, name="b2t")
        ce.dma_start(b2t[:], b2q)
        maskt = consts.tile([P, P], F32, name="maskt")
        ce.dma_start(maskt[:], maskb)

        # Software-pipelined emission: per iteration q the PE stream is
        # [28x W1(q)] [4x W2(q-1)] [tot(q-2), invb(q-2)] so every
        # cross-engine dependency (swish from DVE, exp sums from ACT,
        # reciprocal from DVE) has a quad of slack before the PE needs it.
        hqs, swishes, expos, esums, invcs = {}, {}, {}, {}, {}

        def w1s(q, j, c):
            k = (q * 4 + j) * NCH + c
            return w1t[:, k * Z : (k + 1) * Z]

        def stage1(q):
            """x loads (two halves), W1 matmuls for quad q."""
            hq = hps.tile([P, L], F32, tag="h", name=f"h{q}")
            hqs[q] = hq
            if cfg["x_layout"] == "jp":
                xts = []
                for h in range(2):
                    xt = xpool.tile(
                        [KC, 2 * NCH * L], F8, tag="xt", name=f"xt{q}_{h}"
                    )
                    xes[h % len(xes)].dma_start(xt[:], xq[2 * q + h])
                    xts.append(xt)
                for j in range(4):
                    xt = xts[j // 2]
                    for c in range(NCH):
                        k = (j % 2) * NCH + c
                        nc.tensor.matmul(
                            hq[32 * j : 32 * j + 32, :],
                            w1s(q, j, c),
                            xt[:, k * L : (k + 1) * L],
                            start=(c == 0),
                            stop=(c == NCH - 1),
                            tile_position=(0, 32 * j),
                        )
            else:
                # c-split: free layout (c, j, l); MMs c-outer so the first
                # 16 run off half A. Interleaved accumulation groups on
                # disjoint partition ranges -> skip the group check.
                splits = [(0, 3), (3, 5), (5, 7)] if len(xes) > 2 else [
                    (0, 4), (4, 7)
                ]
                xts = []
                for si, (c0, c1) in enumerate(splits):
                    nchunks = c1 - c0
                    xt = xpool.tile(
                        [KC, nchunks * 4 * L], F8, tag=f"xt{si}", name=f"x{q}_{si}"
                    )
                    xes[si % len(xes)].dma_start(
                        xt[:].rearrange("p (c r) -> p c r", c=nchunks),
                        xq[q, c0:c1].rearrange("c p r -> p c r"),
                    )
                    xts.append(xt)
                for c in range(NCH):
                    si = next(i for i, (c0, c1) in enumerate(splits) if c < c1)
                    xt, cc = xts[si], c - splits[si][0]
                    for j in range(4):
                        nc.tensor.matmul(
                            hq[32 * j : 32 * j + 32, :],
                            w1s(q, j, c),
                            xt[:, (cc * 4 + j) * L : (cc * 4 + j + 1) * L],
                            start=(c == 0),
                            stop=(c == NCH - 1),
                            tile_position=(0, 32 * j),
                            skip_group_check=True,
                        )

        def stage_swish(q):
            """GroupSwish for quad q: ((h+b1)*0.5) * (1 + tanh(sp*(h+b1)/2))."""
            hq = hqs.pop(q)
            t = spool.tile([P, L], F32, tag="t", name=f"t{q}")
            nc.scalar.activation(
                t[:],
                hq[:],
                mybir.ActivationFunctionType.Tanh,
                bias=spb1ht[:, q : q + 1],
                scale=spht[:, q : q + 1],
            )
            u = spool.tile([P, L], F32, tag="u", name=f"u{q}")
            nc.vector.tensor_scalar(
                u[:],
                hq[:],
                b1t[:, q : q + 1],
                0.5,
                op0=mybir.AluOpType.add,
                op1=mybir.AluOpType.mult,
            )
            sw = spool.tile([P, L], F16, tag="sw", name=f"sw{q}")
            nc.vector.scalar_tensor_tensor(
                sw[:],
                t[:],
                1.0,
                u[:],
                op0=mybir.AluOpType.add,
                op1=mybir.AluOpType.mult,
            )
            swishes[q] = sw

        def stage2(q):
            """W2 matmuls + exp for quad q (emitted one quad later)."""
            sw = swishes.pop(q)
            o = ops.tile([P, L], F32, tag="o", name=f"o{q}")
            for j in range(4):
                nc.tensor.matmul(
                    o[32 * j : 32 * j + 32, :],
                    w2t[32 * j : 32 * j + 32, q * 32 : (q + 1) * 32],
                    sw[32 * j : 32 * j + 32, :],
                    start=True,
                    stop=True,
                    tile_position=(32 * j, 32 * j),
                )
            expo = spool.tile([P, L], F32, tag="expo", name=f"e{q}")
            esum = spool.tile([P, 1], F32, tag="esum", name=f"es{q}")
            nc.scalar.activation(
                expo[:],
                o[:],
                mybir.ActivationFunctionType.Exp,
                bias=b2t[:, q : q + 1],
                scale=1.0,
                accum_out=esum[:],
            )
            expos[q] = expo
            esums[q] = esum

        def stage3a(q):
            """Per-group exp totals + reciprocal (two quads later)."""
            esum = esums.pop(q)
            tot = tps.tile([P, 1], F32, tag="tot", name=f"tot{q}")
            nc.tensor.matmul(tot[:], maskt[:], esum[:], start=True, stop=True)
            invc = spool.tile([P, 1], F32, tag="invc", name=f"ic{q}")
            nc.vector.reciprocal(invc[:], tot[:])
            invcs[q] = invc

        def stage3b(q):
            """Normalize + store (two quads later, after stage3a)."""
            invc = invcs.pop(q)
            expo = expos.pop(q)
            res = spool.tile([P, L], BF16, tag="res", name=f"r{q}")
            nc.vector.tensor_scalar_mul(res[:], expo[:], invc[:])
            for j in range(4):
                e = oe if j < 2 else o2e
                e.dma_start(out[4 * q + j], res[32 * j : 32 * j + C, :])

        for q in range(NQ + 2):
            if q < NQ:
                stage1(q)
                stage_swish(q)
            if 1 <= q <= NQ:
                stage2(q - 1)
            if q >= 2:
                stage3a(q - 2)
                stage3b(q - 2)

    nc.compile()
    return nc


def _marshal(x, W1, b1, beta, W2, b2, cfg=DEFAULT_CFG):
    """Full inputs -> list of per-core input dicts."""
    # x: [1, B*X, L] -> [B, 7, 112, L] (g, c, p, l)
    xg = np.asarray(x, dtype=np.float32).reshape(B, NCH, KC, L)
    w1T = np.asarray(W1, dtype=np.float32).transpose(0, 2, 1)  # [B, X, Z]
    w1g = w1T.reshape(B, NCH, KC, Z)  # (g, c, p, z)
    w2s = (np.asarray(W2, dtype=np.float32) * np.float32(1.0 / 1.1))  # [B, C, Z]
    b1f = np.asarray(b1, dtype=np.float32)  # [B, Z]
    b2f = np.asarray(b2, dtype=np.float32)  # [B, C]
    bf = np.asarray(beta, dtype=np.float32)  # [B]
    sph = np.log1p(np.exp(bf)) * np.float32(0.5)  # softplus(beta)/2

    pp = np.arange(P)
    maskb = (
        (pp[:, None] // 32 == pp[None, :] // 32) & (pp[:, None] % 32 < C)
    ).astype(np.float32)

    in_maps = []
    for core in range(NCORE):
        s = slice(core * GPC, (core + 1) * GPC)
        if cfg["x_layout"] == "jp":
            # xq[2q+h, p, j2, c, l] = x[4q+2h+j2, 112c+p, l]
            xc = xg[s].reshape(NQ, 2, 2, NCH, KC, L)
            xqm = (
                xc.transpose(0, 1, 4, 2, 3, 5)
                .astype(ml_dtypes.float8_e4m3)
                .reshape(NQ * 2, KC, 2 * NCH * L)
            )
        else:
            # xq[q, c, p, j, l] = x[4q+j, 112c+p, l]
            xc = xg[s].reshape(NQ, 4, NCH, KC, L)
            xqm = (
                xc.transpose(0, 2, 3, 1, 4)
                .astype(ml_dtypes.float8_e4m3)
                .reshape(NQ, NCH, KC, 4 * L)
            )
        # w1q[p, ((q*4+j)*7+c)*Z+z] = W1T[4q+j, 112c+p, z]
        wc = w1g[s].reshape(NQ, 4, NCH, KC, Z)
        w1qm = (
            wc.transpose(3, 0, 1, 2, 4)
            .astype(ml_dtypes.float8_e4m3)
            .reshape(KC, NQ * 4 * NCH * Z)
        )
        # w2q[32j+z, 32q+m] = W2[4q+j, m, z]/1.1 (m<C), else 0
        w2c = w2s[s].reshape(NQ, 4, C, Z)  # (q, j, m, z)
        w2qm = np.zeros((4, Z, NQ, 32), np.float16)
        w2qm[:, :, :, :C] = w2c.transpose(1, 3, 0, 2)
        w2qm = w2qm.reshape(P, NQ * 32)
        # per-partition scalars: [32j+z, q]
        b1qm = np.ascontiguousarray(
            b1f[s].reshape(NQ, 4, Z).transpose(1, 2, 0)
        ).reshape(P, NQ)
        sphqm = np.ascontiguousarray(
            np.broadcast_to(
                sph[s].reshape(NQ, 4).T[:, None, :], (4, Z, NQ)
            )
        ).reshape(P, NQ)
        spb1hqm = sphqm * b1qm
        b2qm = np.full((4, 32, NQ), -30.0, np.float32)
        b2qm[:, :C, :] = b2f[s].reshape(NQ, 4, C).transpose(1, 2, 0)
        b2qm = b2qm.reshape(P, NQ)
        in_maps.append(
            {
                "xq": xqm,
                "w1q": w1qm,
                "w2q": w2qm,
                "b1q": b1qm,
                "sphq": sphqm,
                "spb1hq": spb1hqm,
                "b2q": b2qm,
                "maskb": maskb,
            }
        )
    return in_maps


def _run(in_maps, cfg=DEFAULT_CFG, trace=False, tmpdir=None):
    key = str(sorted(cfg.items()))
    if key not in _CACHE:
        _CACHE[key] = _build(cfg)
    return run_bass_kernel_spmd(
        _CACHE[key],
        in_maps,
        core_ids=list(range(NCORE)),
        trace=trace,
        tmpdir=tmpdir,
    )


_LAST = {}


def kernel(x, W1, b1, beta, W2, b2):
    cfg = dict(DEFAULT_CFG)
    ov = os.environ.get("KERNEL_CFG")
    if ov:
        for kv in ov.split(","):
            k, v = kv.split("=")
            cfg[k] = type(DEFAULT_CFG[k])(eval(v)) if not isinstance(
                DEFAULT_CFG[k], str
            ) else v
    in_maps = _marshal(x, W1, b1, beta, W2, b2, cfg)
    trace = bool(os.environ.get("KERNEL_TRACE"))
    r = _run(in_maps, cfg, trace=trace, tmpdir=os.environ.get("KERNEL_TRACE_DIR"))
    _LAST["results"] = r
    outs = [
        r.results[c]["out"].astype(np.float32).reshape(GPC, C * L)
        for c in range(NCORE)
    ]
    return np.concatenate(outs, axis=0)


# revision 48
# speedup vs baseline: 1.0639x; 1.0125x over previous
"""Grouped per-sample MLP (conv1d groups=B) + GroupSwish + softmax, on 8 NeuronCores.

Data-parallel over the group/batch axis B=256: 32 groups per core,
processed as 8 quads of 4 groups packed into the 128-partition dim.

Per group g: h = W1[g] @ x[g] + b1[g]; GroupSwish; o = W2[g] @ h + b2[g];
softmax over the flattened [C*L] logits.

The kernel is HBM-stream-bound; every design choice minimizes bytes and
keeps the x stream uninterrupted (~250 GB/s/core under 8-core load):
  - x and W1 ship as fp8e4m3, W2/swish as fp16, out as bf16 (upcast on
    host). End-to-end rel err ~9e-3 vs the 2e-2 gate. 13.3MB/core total.
  - X=784 is split as 7 K-chunks of 112 (uniform K, partitions 0..111).
    Per quad, x is loaded in two DMAs (chunks 0-3 / 4-6, 2KB runs) on two
    queues (sync HWDGE + gpsimd SWDGE) so each SDMA engine keeps two
    packet streams in flight; W1 matmuls start on the first piece.
  - A quad of 4 groups shares each [128, L] tile: group j owns partitions
    32j..32j+32. W1/W2 matmuls are col-tiled via tile_position so the 4
    groups' matmuls run concurrently in the PE array; ACT/DVE ops process
    4 groups per instruction.
  - All of W1 stays resident in SBUF (loaded once) so W1 matmuls gate on
    the x semaphore only.
  - Software-pipelined emission: per iteration q the PE stream is
    [28x W1(q)] [4x W2(q-1)] [tot(q-2)] so every cross-engine dependency
    (swish from DVE, exp sums from ACT, reciprocal from DVE) has a quad
    of slack before the PE needs it; output stores split across two
    queues.
  - W2 is padded to [Z, 32] with zeros so all 128 partitions of the
    logits PSUM are written (pad rows get exp(-30) ~ 0). Softmax
    normalization via one [128,128] block-mask matmul that yields
    per-partition group totals directly.
  - softplus(beta), b1 folding and W2/1.1 folding are done host-side.
"""

import os
import ml_dtypes
import numpy as np
from contextlib import ExitStack

import concourse.mybir as mybir
import concourse.tile as tile
from concourse import bacc
from concourse.bass_utils import run_bass_kernel_spmd

B, X, Z, C, L = 256, 784, 32, 10, 512
NCORE = 8
GPC = B // NCORE  # 32 groups per core
NQ = GPC // 4  # 8 quads per core
KC = 112  # K-chunk size (7 * 112 = 784)
NCH = 7
P = 128
F32 = mybir.dt.float32
F16 = mybir.dt.float16
F8 = mybir.dt.float8e4
BF16 = mybir.dt.bfloat16

DEFAULT_CFG = dict(
    x_bufs=5,
    w_bufs=3,
    s_bufs=4,
    h_bufs=2,
    o_bufs=2,
    x_layout="cc",  # "jp": j-split halves, 14KB descs; "cc": c-split, 2KB descs
    x_engines=("sync", "gpsimd"),
    w_engine="sync",
    out_engine="gpsimd",
    out2_engine="sync",
    const_engine="gpsimd",
)

_CACHE: dict = {}


def _eng(nc, name):
    return getattr(nc, name)


def _build(cfg=DEFAULT_CFG):
    nc = bacc.Bacc("TRN2", target_bir_lowering=False, debug=False)

    # x split into two DMAs per quad so W1 can start on the first half
    # while the second streams. "jp": halves = groups (j01, j23), one 14KB
    # run per partition. "cc": halves = chunks (c0-3, c4-6), 2KB runs.
    if cfg["x_layout"] == "jp":
        xq = nc.dram_tensor(
            "xq", [NQ * 2, KC, 2 * NCH * L], F8, kind="ExternalInput"
        ).ap()
    else:
        xq = nc.dram_tensor(
            "xq", [NQ, NCH, KC, 4 * L], F8, kind="ExternalInput"
        ).ap()
    w1q = nc.dram_tensor(
        "w1q", [KC, NQ * 4 * NCH * Z], F8, kind="ExternalInput"
    ).ap()
    # w2q[32j+z, 32q+m] = W2[4q+j, m, z]/1.1 (m<C), 0 for m>=C
    w2q = nc.dram_tensor("w2q", [P, NQ * 32], F16, kind="ExternalInput").ap()
    b1q = nc.dram_tensor("b1q", [P, NQ], F32, kind="ExternalInput").ap()
    sphq = nc.dram_tensor("sphq", [P, NQ], F32, kind="ExternalInput").ap()
    spb1hq = nc.dram_tensor("spb1hq", [P, NQ], F32, kind="ExternalInput").ap()
    b2q = nc.dram_tensor("b2q", [P, NQ], F32, kind="ExternalInput").ap()
    # maskb[p, m] = 1 iff p//32 == m//32 and p%32 < C: one matmul turns the
    # per-partition exp sums into per-partition group totals (pad rows get
    # the same total, keeping reciprocal finite).
    maskb = nc.dram_tensor("maskb", [P, P], F32, kind="ExternalInput").ap()
    out = nc.dram_tensor("out", [GPC, C, L], BF16, kind="ExternalOutput").ap()

    with tile.TileContext(nc) as tc, ExitStack() as ctx:
        consts = ctx.enter_context(tc.tile_pool(name="consts", bufs=1))
        xpool = ctx.enter_context(tc.tile_pool(name="x", bufs=2 * cfg["x_bufs"]))
        spool = ctx.enter_context(tc.tile_pool(name="act", bufs=cfg["s_bufs"]))
        hps = ctx.enter_context(
            tc.tile_pool(name="hps", bufs=cfg["h_bufs"], space="PSUM")
        )
        ops = ctx.enter_context(
            tc.tile_pool(name="ops", bufs=cfg["o_bufs"], space="PSUM")
        )
        tps = ctx.enter_context(tc.tile_pool(name="tps", bufs=2, space="PSUM"))

        ce = _eng(nc, cfg["const_engine"])
        xes = [_eng(nc, e) for e in cfg["x_engines"]]
        we = _eng(nc, cfg["w_engine"])
        oe = _eng(nc, cfg["out_engine"])
        o2e = _eng(nc, cfg["out2_engine"])

        # all of W1 stays resident (14.3KB/partition) -> W1 matmuls gate on
        # the x DMA semaphore only
        w1t = consts.tile([KC, NQ * 4 * NCH * Z], F8, name="w1t")
        we.dma_start(w1t[:], w1q)
        w2t = consts.tile([P, NQ * 32], F16, name="w2t")
        ce.dma_start(w2t[:], w2q)
        b1t = consts.tile([P, NQ], F32, name="b1t")
        ce.dma_start(b1t[:], b1q)
        spht = consts.tile([P, NQ], F32, name="spht")
        ce.dma_start(spht[:], sphq)
        spb1ht = consts.tile([P, NQ], F32, name="spb1ht")
        ce.dma_start(spb1ht[:], spb1hq)
        b2t = consts.tile([P, NQ], F32, name="b2t")
        ce.dma_start(b2t[:], b2q)
        maskt = consts.tile([P, P], F32, name="maskt")
        ce.dma_start(maskt[:], maskb)

        # Software-pipelined emission: per iteration q the PE stream is
        # [28x W1(q)] [4x W2(q-1)] [tot(q-2), invb(q-2)] so every
        # cross-engine dependency (swish from DVE, exp sums from ACT,
        # reciprocal from DVE) has a quad of slack before the PE needs it.
        hqs, swishes, expos, esums, invcs = {}, {}, {}, {}, {}

        def w1s(q, j, c):
            k = (q * 4 + j) * NCH + c
            return w1t[:, k * Z : (k + 1) * Z]

        def stage1(q):
            """x loads (two halves), W1 matmuls for quad q."""
            hq = hps.tile([P, L], F32, tag="h", name=f"h{q}")
            hqs[q] = hq
            if cfg["x_layout"] == "jp":
                xts = []
                for h in range(2):
                    xt = xpool.tile(
                        [KC, 2 * NCH * L], F8, tag="xt", name=f"xt{q}_{h}"
                    )
                    xes[h % len(xes)].dma_start(xt[:], xq[2 * q + h])
                    xts.append(xt)
                for j in range(4):
                    xt = xts[j // 2]
                    for c in range(NCH):
                        k = (j % 2) * NCH + c
                        nc.tensor.matmul(
                            hq[32 * j : 32 * j + 32, :],
                            w1s(q, j, c),
                            xt[:, k * L : (k + 1) * L],
                            start=(c == 0),
                            stop=(c == NCH - 1),
                            tile_position=(0, 32 * j),
                        )
            else:
                # c-split: free layout (c, j, l); MMs c-outer so the first
                # 16 run off half A. Interleaved accumulation groups on
                # disjoint partition ranges -> skip the group check.
                if q == NQ - 1:
                    splits = [(0, 4), (4, 6), (6, 7)]
                else:
                    splits = [(0, 4), (4, 7)]
                xts = []
                for si, (c0, c1) in enumerate(splits):
                    nchunks = c1 - c0
                    xt = xpool.tile(
                        [KC, nchunks * 4 * L], F8, tag=f"xt{si}", name=f"x{q}_{si}"
                    )
                    xes[si % len(xes)].dma_start(
                        xt[:].rearrange("p (c r) -> p c r", c=nchunks),
                        xq[q, c0:c1].rearrange("c p r -> p c r"),
                    )
                    xts.append(xt)
                for c in range(NCH):
                    si = next(i for i, (c0, c1) in enumerate(splits) if c < c1)
                    xt, cc = xts[si], c - splits[si][0]
                    for j in range(4):
                        nc.tensor.matmul(
                            hq[32 * j : 32 * j + 32, :],
                            w1s(q, j, c),
                            xt[:, (cc * 4 + j) * L : (cc * 4 + j + 1) * L],
                            start=(c == 0),
                            stop=(c == NCH - 1),
                            tile_position=(0, 32 * j),
                            skip_group_check=True,
                        )

        def stage_swish(q):
            """GroupSwish for quad q: ((h+b1)*0.5) * (1 + tanh(sp*(h+b1)/2))."""
            hq = hqs.pop(q)
            t = spool.tile([P, L], F32, tag="t", name=f"t{q}")
            nc.scalar.activation(
                t[:],
                hq[:],
                mybir.ActivationFunctionType.Tanh,
                bias=spb1ht[:, q : q + 1],
                scale=spht[:, q : q + 1],
            )
            u = spool.tile([P, L], F32, tag="u", name=f"u{q}")
            nc.vector.tensor_scalar(
                u[:],
                hq[:],
                b1t[:, q : q + 1],
                0.5,
                op0=mybir.AluOpType.add,
                op1=mybir.AluOpType.mult,
            )
            sw = spool.tile([P, L], F16, tag="sw", name=f"sw{q}")
            nc.vector.scalar_tensor_tensor(
                sw[:],
                t[:],
                1.0,
                u[:],
                op0=mybir.AluOpType.add,
                op1=mybir.AluOpType.mult,
            )
            swishes[q] = sw

        def stage2(q):
            """W2 matmuls + exp for quad q (emitted one quad later)."""
            sw = swishes.pop(q)
            o = ops.tile([P, L], F32, tag="o", name=f"o{q}")
            for j in range(4):
                nc.tensor.matmul(
                    o[32 * j : 32 * j + 32, :],
                    w2t[32 * j : 32 * j + 32, q * 32 : (q + 1) * 32],
                    sw[32 * j : 32 * j + 32, :],
                    start=True,
                    stop=True,
                    tile_position=(32 * j, 32 * j),
                )
            expo = spool.tile([P, L], F32, tag="expo", name=f"e{q}")
            esum = spool.tile([P, 1], F32, tag="esum", name=f"es{q}")
            nc.scalar.activation(
                expo[:],
                o[:],
                mybir.ActivationFunctionType.Exp,
                bias=b2t[:, q : q + 1],
                scale=1.0,
                accum_out=esum[:],
            )
            expos[q] = expo
            esums[q] = esum

        def stage3a(q):
            """Per-group exp totals + reciprocal (two quads later)."""
            esum = esums.pop(q)
            tot = tps.tile([P, 1], F32, tag="tot", name=f"tot{q}")
            nc.tensor.matmul(tot[:], maskt[:], esum[:], start=True, stop=True)
            invc = spool.tile([P, 1], F32, tag="invc", name=f"ic{q}")
            nc.vector.reciprocal(invc[:], tot[:])
            invcs[q] = invc

        def stage3b(q):
            """Normalize + store (two quads later, after stage3a)."""
            invc = invcs.pop(q)
            expo = expos.pop(q)
            res = spool.tile([P, L], BF16, tag="res", name=f"r{q}")
            nc.vector.tensor_scalar_mul(res[:], expo[:], invc[:])
            for j in range(4):
                e = oe if j < 2 else o2e
                e.dma_start(out[4 * q + j], res[32 * j : 32 * j + C, :])

        for q in range(NQ + 2):
            if q < NQ:
                stage1(q)
                stage_swish(q)
            if 1 <= q <= NQ:
                stage2(q - 1)
            if q >= 2:
                stage3a(q - 2)
                stage3b(q - 2)

    nc.compile()
    return nc


def _marshal(x, W1, b1, beta, W2, b2, cfg=DEFAULT_CFG):
    """Full inputs -> list of per-core input dicts."""
    # x: [1, B*X, L] -> [B, 7, 112, L] (g, c, p, l)
    xg = np.asarray(x, dtype=np.float32).reshape(B, NCH, KC, L)
    w1T = np.asarray(W1, dtype=np.float32).transpose(0, 2, 1)  # [B, X, Z]
    w1g = w1T.reshape(B, NCH, KC, Z)  # (g, c, p, z)
    w2s = (np.asarray(W2, dtype=np.float32) * np.float32(1.0 / 1.1))  # [B, C, Z]
    b1f = np.asarray(b1, dtype=np.float32)  # [B, Z]
    b2f = np.asarray(b2, dtype=np.float32)  # [B, C]
    bf = np.asarray(beta, dtype=np.float32)  # [B]
    sph = np.log1p(np.exp(bf)) * np.float32(0.5)  # softplus(beta)/2

    pp = np.arange(P)
    maskb = (
        (pp[:, None] // 32 == pp[None, :] // 32) & (pp[:, None] % 32 < C)
    ).astype(np.float32)

    in_maps = []
    for core in range(NCORE):
        s = slice(core * GPC, (core + 1) * GPC)
        if cfg["x_layout"] == "jp":
            # xq[2q+h, p, j2, c, l] = x[4q+2h+j2, 112c+p, l]
            xc = xg[s].reshape(NQ, 2, 2, NCH, KC, L)
            xqm = (
                xc.transpose(0, 1, 4, 2, 3, 5)
                .astype(ml_dtypes.float8_e4m3)
                .reshape(NQ * 2, KC, 2 * NCH * L)
            )
        else:
            # xq[q, c, p, j, l] = x[4q+j, 112c+p, l]
            xc = xg[s].reshape(NQ, 4, NCH, KC, L)
            xqm = (
                xc.transpose(0, 2, 3, 1, 4)
                .astype(ml_dtypes.float8_e4m3)
                .reshape(NQ, NCH, KC, 4 * L)
            )
        # w1q[p, ((q*4+j)*7+c)*Z+z] = W1T[4q+j, 112c+p, z]
        wc = w1g[s].reshape(NQ, 4, NCH, KC, Z)
        w1qm = (
            wc.transpose(3, 0, 1, 2, 4)
            .astype(ml_dtypes.float8_e4m3)
            .reshape(KC, NQ * 4 * NCH * Z)
        )
        # w2q[32j+z, 32q+m] = W2[4q+j, m, z]/1.1 (m<C), else 0
        w2c = w2s[s].reshape(NQ, 4, C, Z)  # (q, j, m, z)
        w2qm = np.zeros((4, Z, NQ, 32), np.float16)
        w2qm[:, :, :, :C] = w2c.transpose(1, 3, 0, 2)
        w2qm = w2qm.reshape(P, NQ * 32)
        # per-partition scalars: [32j+z, q]
        b1qm = np.ascontiguousarray(
            b1f[s].reshape(NQ, 4, Z).transpose(1, 2, 0)
        ).reshape(P, NQ)
        sphqm = np.ascontiguousarray(
            np.broadcast_to(
                sph[s].reshape(NQ, 4).T[:, None, :], (4, Z, NQ)
            )
        ).reshape(P, NQ)
        spb1hqm = sphqm * b1qm
        b2qm = np.full((4, 32, NQ), -30.0, np.float32)
        b2qm[:, :C, :] = b2f[s].reshape(NQ, 4, C).transpose(1, 2, 0)
        b2qm = b2qm.reshape(P, NQ)
        in_maps.append(
            {
                "xq": xqm,
                "w1q": w1qm,
                "w2q": w2qm,
                "b1q": b1qm,
                "sphq": sphqm,
                "spb1hq": spb1hqm,
                "b2q": b2qm,
                "maskb": maskb,
            }
        )
    return in_maps


def _run(in_maps, cfg=DEFAULT_CFG, trace=False, tmpdir=None):
    key = str(sorted(cfg.items()))
    if key not in _CACHE:
        _CACHE[key] = _build(cfg)
    return run_bass_kernel_spmd(
        _CACHE[key],
        in_maps,
        core_ids=list(range(NCORE)),
        trace=trace,
        tmpdir=tmpdir,
    )


_LAST = {}


def kernel(x, W1, b1, beta, W2, b2):
    cfg = dict(DEFAULT_CFG)
    ov = os.environ.get("KERNEL_CFG")
    if ov:
        for kv in ov.split(","):
            k, v = kv.split("=")
            cfg[k] = type(DEFAULT_CFG[k])(eval(v)) if not isinstance(
                DEFAULT_CFG[k], str
            ) else v
    in_maps = _marshal(x, W1, b1, beta, W2, b2, cfg)
    trace = bool(os.environ.get("KERNEL_TRACE"))
    r = _run(in_maps, cfg, trace=trace, tmpdir=os.environ.get("KERNEL_TRACE_DIR"))
    _LAST["results"] = r
    outs = [
        r.results[c]["out"].astype(np.float32).reshape(GPC, C * L)
        for c in range(NCORE)
    ]
    return np.concatenate(outs, axis=0)
